# revision 52
# baseline (speedup 1.0000x reference)
"""Trainium2 Bass kernel for AttentionStem (sparse local 4x4-window attention).

Contract: kernel(**inputs) takes the FULL unsharded inputs (numpy, keyed as in
setup_inputs()) and returns the FULL output [4, 64, 128, 128] float32.

Algorithm (per output pixel (b, h, w), per channel o):
    q  = query_w @ x                    (1x1 conv)
    kc = key_w @ xpad                   (1x1 conv on padded grid)
    vs_k = W_k @ xpad,  W_k = sum_m softmax_m(emb)[m,k] * value_w[m]
    attn_k = softmax_k(q * kc[pix + off_k])        (16 window positions)
    out = sum_k attn_k * vs_k[pix + off_k]

Sharding: 8 cores = 4 batches x 2 H-halves (64 output rows each, 4-row halo).
Layout on chip: [128 partitions = 2 row-blocks x 64 channels, free = pixels]
with padded row stride 132 so every window shift is a contiguous slice.
Softmax is computed without max subtraction: |logit| <= |q|*|k| stays well
below exp overflow for these Gaussian-scaled inputs.

Precision strategy: the logit convs (q, kc) run in fp32 on the TensorEngine
(exp is sensitive to absolute logit error); everything downstream runs in
bf16 on the 2x-mode VectorE path (fp32 PSUM accumulation inside each matmul,
split bf16 accumulator slots across the 16 window positions). Window
positions are processed in vertical pairs (i, i+1) fused into single
VectorE/ScalarE ops via overlapping / broadcast access patterns.
Measured absmax rel err vs the fp32 reference: ~1.2e-2 (gate 2e-2).
Measured HW exec time: ~189 us on NeuronCore 0 (neuron-profile NTFF).
"""

import os
import sys

import numpy as np

sys.path.insert(0, "/opt/trn_rl_repo")

# Problem constants (hardcoded; kernel.py must be self-contained).
B, IC, OC, H, W = 4, 3, 64, 128, 128
KS, PAD, M = 4, 2, 4
NCORES = 8

W132 = W + 2 * PAD  # padded width = 132
SH_OUT_ROWS = 64  # output rows per core
SH_ROWS = SH_OUT_ROWS + KS  # padded input rows per core = 68
XP_FREE = SH_ROWS * W132 + 16  # xp slab free size (+pad for shifted reads)
BLK = 32  # output rows per partition-block
NBLK_FREE = BLK * W132  # 4224 free elems per block
KCV_ROWS = BLK + KS - 1  # 35 conv rows needed per block
KCV_FREE = KCV_ROWS * W132  # 4620
KCV_PAD = 16

# Config knobs (tuned on hardware).
CFG = {
    "conv_qk": os.environ.get("K_CONV_QK", "f32"),  # logit conv matmul dtype
    "conv_vs": os.environ.get("K_CONV_VS", "bf16"),  # value conv matmul dtype
    "el": os.environ.get("K_EL", "bf16"),  # elementwise dtype (L/e/p/q/kcv)
    "acc": os.environ.get("K_ACC", "bf16"),  # s/num accumulator dtype
    "half": int(os.environ.get("K_HALF", "2")),  # column-split factor
    "gpadd": os.environ.get("K_GPADD", "1") == "1",  # split (ping-pong) accumulators
}

_CACHE = {}
LAST_RESULT = None  # BassKernelResults of the most recent run (for test.py)


def _dt(name):
    from concourse import mybir

    return {
        "f32": mybir.dt.float32,
        "f32r": mybir.dt.float32r,
        "bf16": mybir.dt.bfloat16,
    }[name]


def _emit(nc, tc, aps, cfg):
    """Emit the per-core program.

    aps: dict with xpf/xpb [3, XP_FREE], wtsf/wtsb [3, 64*18], out [64,64,128].
    wts layout: [qw.T | kw.T | W_0.T .. W_15.T]."""
    from contextlib import ExitStack

    import concourse.bass as bass
    from concourse import mybir

    f32 = mybir.dt.float32
    eldt = _dt(cfg["el"])
    adt = _dt(cfg["acc"])
    EXP = mybir.ActivationFunctionType.Exp
    NH = cfg["half"]
    HF = NBLK_FREE // NH  # elementwise free size per iteration
    el_bf16 = cfg["el"] == "bf16"

    with ExitStack() as ctx:
        const = ctx.enter_context(tc.tile_pool(name="const", bufs=1))
        qkp = ctx.enter_context(tc.tile_pool(name="qk", bufs=1))

        # ---- load inputs ----
        need_f32 = "f32" in (cfg["conv_qk"], cfg["conv_vs"]) or cfg["conv_qk"] == "f32r"
        need_bf16 = "bf16" in (cfg["conv_qk"], cfg["conv_vs"])
        xp = {}
        wts = {}
        # load order: weights, then the xp rows both row-blocks touch first
        # (block0 rows 0-23, block1 rows 32-55), then the remainder.
        P1A, P1B0, P1B1 = 12 * W132, 32 * W132, 44 * W132
        if need_f32:
            xp["f32"] = const.tile([IC, XP_FREE], f32, tag="xpf32", name="xpf32")
            wts["f32"] = const.tile([IC, OC * 18], f32, tag="wtsf32", name="wtsf32")
            xp["f32r"] = xp["f32"].bitcast(mybir.dt.float32r)
            wts["f32r"] = wts["f32"].bitcast(mybir.dt.float32r)
        if need_bf16:
            xp["bf16"] = const.tile([IC, XP_FREE], mybir.dt.bfloat16, tag="xpbf", name="xpbf")
            wts["bf16"] = const.tile([IC, OC * 18], mybir.dt.bfloat16, tag="wtsbf", name="wtsbf")
        # The logit-conv critical path (wtsf + first xpf pieces) gets the
        # sync queue with minimal descriptors ahead of it; the bf16 side
        # issues in parallel from the (idle at t=0) ScalarE/VectorE queues.
        if need_f32:
            nc.sync.dma_start(wts["f32"][:], aps["wtsf"][:])
        if need_bf16:
            nc.scalar.dma_start(wts["bf16"][:], aps["wtsb"][:])
        for lo, hi in ((0, P1A), (P1B0, P1B1), (P1A, P1B0), (P1B1, XP_FREE)):
            if need_f32:
                nc.sync.dma_start(xp["f32"][:, lo:hi], aps["xpf"][:, lo:hi])
            if need_bf16:
                nc.scalar.dma_start(xp["bf16"][:, lo:hi], aps["xpb"][:, lo:hi])

        def conv_rows(psum_pool, dst, wslot, dtname, base0, base1, total,
                      dst_off=0, chunk=1024, evac=None):
            """dst[128, f+dst_off] = sum_c w[o,c] * xp[c, base_blk + f]
            for partition 64*blk + o, f in [0, total). Chunked + evacuated."""
            xp_s = xp[dtname]
            w_l = wts[dtname][:, OC * wslot : OC * (wslot + 1)]
            off = 0
            while off < total:
                n = min(chunk, total - off)
                pt = psum_pool.tile([128, 1024], f32, tag="convp", name="cp")
                coff = 0
                while coff < n:
                    cn = min(512, n - coff)
                    for b, base in ((0, base0), (1, base1)):
                        nc.tensor.matmul(
                            pt[64 * b : 64 * (b + 1), coff : coff + cn],
                            w_l,
                            xp_s[:, base + off + coff : base + off + coff + cn],
                        )
                    coff += cn
                # evacuate PSUM -> SBUF (ScalarE: close to PSUM; casts dtype)
                (evac or nc.scalar.copy)(
                    dst[:, dst_off + off : dst_off + off + n], pt[:, :n]
                )
                off += n

        def conv_rows_compact(psum_pool, dst, wslot, dtname, base0, base1,
                              nrows_tot, dst_off=0):
            """Like conv_rows but skips the 4 pad columns per row: the matmul
            rhs uses a [rows,132->128] strided view, PSUM and dst stay
            contiguous at 128/row."""
            xp_s = xp[dtname]
            w_l = wts[dtname][:, OC * wslot : OC * (wslot + 1)]
            r = 0
            while r < nrows_tot:
                nr_c = min(8, nrows_tot - r)
                pt = psum_pool.tile([128, 1024], f32, tag="convp", name="cp")
                rr = 0
                while rr < nr_c:
                    sub = min(4, nr_c - rr)
                    for b, base in ((0, base0), (1, base1)):
                        rhs = xp_s[:, base + (r + rr) * W132 :
                                   base + (r + rr + sub) * W132]
                        rhs3 = rhs.rearrange("c (r w) -> c r w",
                                             w=W132)[:, :, 0:W]
                        nc.tensor.matmul(
                            pt[64 * b : 64 * (b + 1),
                               rr * W : (rr + sub) * W],
                            w_l, rhs3,
                        )
                    rr += sub
                nc.scalar.copy(
                    dst[:, dst_off + r * W : dst_off + (r + nr_c) * W],
                    pt[:, : nr_c * W],
                )
                r += nr_c

        # ---- q/kcv tiles (filled per column-half so the second half's convs
        # overlap the first half's k-loop instead of serializing up front) ----
        q = qkp.tile([128, NBLK_FREE], eldt, tag="q")
        # kcv with 1-element shifted twin so both shift parities have
        # 4B-aligned reads (keeps DVE 2x mode).
        kcv0 = qkp.tile([128, KCV_FREE + KCV_PAD], eldt, tag="kcv0")
        if el_bf16:
            kcv1 = qkp.tile([128, KCV_FREE + KCV_PAD], eldt, tag="kcv1")
        else:
            kcv1 = None
        # Sections of block-rows: a small first section so the first logit
        # multiply starts after ~1/4 of the phase-0 conv work; later sections'
        # q/kcv convs hide under the previous section's k-loop.
        # (row0, nrows, kcv_row_lo, kcv_row_hi)
        if NH == 2:
            SECTIONS = [(0, 4, 0, 8), (4, 12, 8, 20), (16, 16, 20, KCV_ROWS)]
        else:
            SECTIONS = [(0, BLK, 0, KCV_ROWS)]

        def qk_phase(sec, psum_pool):
            row0, nrows, klo_r, khi_r = SECTIONS[sec]
            qlo, qhi = row0 * W132, (row0 + nrows) * W132
            klo, khi = klo_r * W132, khi_r * W132
            ev = nc.vector.tensor_copy if sec == 0 else None
            # q: output rows b*32+h, cols w -> xp free (b*32+h+2)*132 + (w+2)
            conv_rows(
                psum_pool, q, 0, cfg["conv_qk"],
                (0 * BLK + PAD) * W132 + PAD + qlo,
                (1 * BLK + PAD) * W132 + PAD + qlo,
                qhi - qlo, dst_off=qlo, chunk=1024, evac=ev,
            )
            # kcv: conv at padded rows [b*32, b*32+35)
            conv_rows(
                psum_pool, kcv0, 1, cfg["conv_qk"],
                (0 * BLK) * W132 + klo,
                (1 * BLK) * W132 + klo,
                khi - klo, dst_off=klo, chunk=1024, evac=ev,
            )
            last = sec == len(SECTIONS) - 1
            if last:
                nc.vector.memset(kcv0[:, KCV_FREE:], 0.0)
            if kcv1 is not None:
                # kcv1[f] = kcv0[f+1]; section boundaries at khi*132 - 1
                lo = klo - 1 if klo > 0 else 0
                hi = (KCV_FREE + KCV_PAD - 8) if last else khi - 1
                if sec == 0:
                    # ramp window: VectorE is idle anyway
                    nc.vector.tensor_copy(kcv1[:, lo:hi], kcv0[:, lo + 1 : hi + 1])
                else:
                    # steady state: ride the idle DMA engines, split across
                    # partition quarters for bandwidth
                    for p0 in range(0, 128, 32):
                        nc.sync.dma_start(
                            kcv1[p0 : p0 + 32, lo:hi],
                            kcv0[p0 : p0 + 32, lo + 1 : hi + 1],
                        )
                if last:
                    nc.vector.memset(kcv1[:, KCV_FREE:], 0.0)

        # ---- phase 1: 16-way softmax-weighted accumulation ----
        # Software-pipelined: L_{k+1} is emitted while ACT runs exp_k so the
        # VectorE never waits on the ScalarE. e_k and p_k share one [128,2*HF]
        # tile (e left, p right) so the s/num accumulation is a single add.
        with ExitStack() as ctx1:
            vsp = ctx1.enter_context(tc.tile_pool(name="vs", bufs=2))
            psum1 = ctx1.enter_context(
                tc.tile_pool(name="psum1", bufs=4, space="PSUM")
            )
            epp = ctx1.enter_context(tc.tile_pool(name="ep", bufs=2))
            accp = ctx1.enter_context(tc.tile_pool(name="acc", bufs=2))
            finp = ctx1.enter_context(tc.tile_pool(name="fin", bufs=1))
            outp = ctx1.enter_context(tc.tile_pool(name="out", bufs=2))

            NK = KS * KS
            # vertical window pairs (i, j) & (i+1, j): kcv shifts differ by
            # exactly one padded row (132 elements - even, so bf16 alignment
            # and the 2x VectorE mode survive the paired access pattern)
            PAIRS = [(0, 4), (1, 5), (2, 6), (3, 7),
                     (8, 12), (9, 13), (10, 14), (11, 15)]

            def vs_pair(ka, kb, hoff, nrows):
                hfc = nrows * W  # compact (128/row) slot size
                v2 = vsp.tile([128, 2 * hfc], eldt, tag="vs", name="vs")
                for idx, k in ((0, ka), (1, kb)):
                    i, j = k // KS, k % KS
                    conv_rows_compact(
                        psum1, v2, 2 + k, cfg["conv_vs"],
                        (0 * BLK + i) * W132 + j + hoff,
                        (1 * BLK + i) * W132 + j + hoff,
                        nrows, dst_off=idx * hfc,
                    )
                return v2

            def sv(ap, off, nrows):
                """[128, nrows*132] slice viewed as [128, nrows, 128]: skips
                the 4 pad columns per row (3% fewer elements per DVE/ACT op;
                inner dim stays step-1 and 4B-aligned so 2x mode holds)."""
                v = ap[:, off : off + nrows * W132]
                return v.rearrange("p (r w) -> p r w", w=W132)[:, :, 0:W]

            def logit_pair(ka, hoff, hf, dst):
                """One op computes L_ka and L_{ka+4} into slots 0/1 of dst:
                the kcv operand uses a [132, 2] outer access dim (one row
                apart), the q operand a step-0 broadcast dim."""
                i, j = ka // KS, ka % KS
                shift = i * W132 + j
                if kcv1 is not None and (shift % 2) == 1:
                    ksrc, koff = kcv1, shift - 1
                else:
                    ksrc, koff = kcv0, shift
                nr = hf // W132
                vk = sv(ksrc, koff + hoff, nr)
                kk = bass.AP(vk.tensor, vk.offset,
                             [list(vk.ap)[0], [W132, 2], *list(vk.ap)[1:]])
                vq = sv(q, hoff, nr)
                qq = bass.AP(vq.tensor, vq.offset,
                             [list(vq.ap)[0], [0, 2], *list(vq.ap)[1:]])
                out = dst[:, 0 : 2 * hf].rearrange(
                    "p (s r w) -> p s r w", s=2, w=W132)[:, :, :, 0:W]
                nc.vector.tensor_mul(out, qq, kk)

            # Pair-fused k-loop: window positions processed two at a time in
            # one [128, 4*hf] tile laid out [e0|e1|p0|p1]. The logit multiply,
            # exp, value multiply, and accumulate each run once per pair
            # (half the per-op overheads), and the slot split doubles as the
            # accuracy "ping-pong".
            for sec in range(len(SECTIONS)):
                row0, nrows, _, _ = SECTIONS[sec]
                hoff, hf = row0 * W132, nrows * W132
                nr = nrows
                if sec == 0:
                    qk_phase(0, psum1)
                acc4 = accp.tile([128, 4 * hf], adt, tag="acc4", name="acc4")

                ep = epp.tile([128, 4 * hf], eldt, tag="ep", name="ep")
                vs2 = vs_pair(PAIRS[0][0], PAIRS[0][1], hoff, nrows)
                logit_pair(PAIRS[0][0], hoff, hf, ep)
                for t in range(NK // 2):
                    nc.scalar.activation(sv(ep, 0, 2 * nr), sv(ep, 0, 2 * nr),
                                         EXP)
                    nc.vector.tensor_mul(
                        sv(ep, 2 * hf, 2 * nr), sv(ep, 0, 2 * nr),
                        vs2[:, 0 : 2 * nr * W].rearrange(
                            "p (r w) -> p r w", w=W),
                    )
                    if t == (3 if sec == 0 else 6) and sec + 1 < len(SECTIONS):
                        # next section's q/kcv convs: emitted mid-loop so the
                        # TensorE has them ready before the section boundary
                        qk_phase(sec + 1, psum1)
                    ep_prev = ep
                    if t + 1 < NK // 2:
                        ep = epp.tile([128, 4 * hf], eldt, tag="ep", name="ep")
                        vs2 = vs_pair(PAIRS[t + 1][0], PAIRS[t + 1][1],
                                      hoff, nrows)
                        logit_pair(PAIRS[t + 1][0], hoff, hf, ep)
                    if t == 0:
                        nc.vector.tensor_copy(sv(acc4, 0, 4 * nr),
                                              sv(ep_prev, 0, 4 * nr))
                    else:
                        nc.vector.tensor_add(sv(acc4, 0, 4 * nr),
                                             sv(acc4, 0, 4 * nr),
                                             sv(ep_prev, 0, 4 * nr))

                # fold the even/odd slots: acc[s|num] = slots(0,2) + slots(1,3)
                acc = accp.tile([128, 2 * hf], adt, tag="acc", name="acc")
                a4 = acc4[:].rearrange("p (g s r w) -> p g s r w", g=2, s=2,
                                       w=W132)[:, :, :, :, 0:W]
                a2 = acc[:].rearrange("p (g r w) -> p g r w", g=2,
                                      w=W132)[:, :, :, 0:W]
                nc.vector.tensor_add(a2, a4[:, :, 0], a4[:, :, 1])

                # out = num / s  (s needs fp32 for the bit-level recip seed)
                # in column pieces so the store DMA overlaps the final mults;
                # compacted to the 128 valid columns per row
                npiece = 2 if nrows >= 8 else 1
                rpp = nrows // npiece  # block rows per final piece
                FP = rpp * W  # compact piece size
                for piece in range(npiece):
                    plo = piece * rpp * W132
                    s_f = finp.tile([128, FP], f32, tag="sf", name="sf")
                    nc.scalar.copy(s_f[:], sv(acc, plo, rpp))
                    rinv = finp.tile([128, FP], f32, tag="rinv", name="rinv")
                    nc.vector.reciprocal_approx_fast(rinv[:], s_f[:])
                    o_t = outp.tile([128, FP], f32, tag="o", name="o")
                    nc.vector.tensor_mul(
                        o_t[:], sv(acc, hf + plo, rpp),
                        rinv[:].rearrange("p (r w) -> p r w", w=W),
                    )
                    o_v = o_t[:].rearrange("p (r w) -> p r w", w=W)
                    rr = row0 + piece * rpp
                    for b in (0, 1):
                        nc.sync.dma_start(
                            aps["out"][:, b * BLK + rr : b * BLK + rr + rpp, :],
                            o_v[64 * b : 64 * (b + 1)],
                        )


def _build(cfg):
    key = tuple(sorted(cfg.items()))
    if key in _CACHE:
        return _CACHE[key]
    import concourse.tile as tile
    from concourse import bacc, mybir

    nc = bacc.Bacc(
        "TRN2", target_bir_lowering=False, debug=False, num_devices=NCORES
    )
    f32 = mybir.dt.float32
    bf16 = mybir.dt.bfloat16
    aps = {}
    need_f32 = "f32" in (cfg["conv_qk"], cfg["conv_vs"]) or cfg["conv_qk"] == "f32r"
    need_bf16 = "bf16" in (cfg["conv_qk"], cfg["conv_vs"])
    if need_f32:
        aps["xpf"] = nc.dram_tensor("xpf", [IC, XP_FREE], f32,
                                    kind="ExternalInput").ap()
        aps["wtsf"] = nc.dram_tensor("wtsf", [IC, OC * 18], f32,
                                     kind="ExternalInput").ap()
    if need_bf16:
        aps["xpb"] = nc.dram_tensor("xpb", [IC, XP_FREE], bf16,
                                    kind="ExternalInput").ap()
        aps["wtsb"] = nc.dram_tensor("wtsb", [IC, OC * 18], bf16,
                                     kind="ExternalInput").ap()
    aps["out"] = nc.dram_tensor("out", [OC, SH_OUT_ROWS, W], f32,
                                kind="ExternalOutput").ap()

    with tile.TileContext(nc) as tc:
        _emit(nc, tc, aps, cfg)
    nc.compile()
    _CACHE[key] = nc
    return nc


def _host_prep(inputs, cfg):
    import ml_dtypes

    x = np.asarray(inputs["x"], np.float32)
    key_w = np.asarray(inputs["key_w"], np.float32)
    query_w = np.asarray(inputs["query_w"], np.float32)
    value_w = np.asarray(inputs["value_w"], np.float32)
    emb_a = np.asarray(inputs["emb_a"], np.float32)
    emb_b = np.asarray(inputs["emb_b"], np.float32)
    emb_mix = np.asarray(inputs["emb_mix"], np.float32)

    # emb softmax over m, then effective per-offset value matrices W_k [16,64,3]
    la = emb_mix @ emb_a  # (M, KS)
    lb = emb_mix @ emb_b  # (M, KS)
    eloG = (la[:, :, None] + lb[:, None, :]).reshape(M, KS * KS).astype(np.float64)
    eloG -= eloG.max(axis=0, keepdims=True)
    emb = np.exp(eloG)
    emb /= emb.sum(axis=0, keepdims=True)  # (M, 16)
    wk = np.einsum("mk,moc->koc", emb.astype(np.float32), value_w)  # (16,64,3)

    # weights tensor [3, 64*18] = [qw.T | kw.T | W_0.T .. W_15.T]
    wts = np.empty((IC, OC * 18), np.float32)
    wts[:, 0:OC] = query_w.T
    wts[:, OC : 2 * OC] = key_w.T
    for k in range(KS * KS):
        wts[:, OC * (2 + k) : OC * (3 + k)] = wk[k].T

    # padded input, shards
    xp = np.zeros((B, IC, H + 2 * PAD, W + 2 * PAD), np.float32)
    xp[:, :, PAD : PAD + H, PAD : PAD + W] = x

    need_f32 = "f32" in (cfg["conv_qk"], cfg["conv_vs"]) or cfg["conv_qk"] == "f32r"
    need_bf16 = "bf16" in (cfg["conv_qk"], cfg["conv_vs"])
    wtsb = wts.astype(ml_dtypes.bfloat16)

    in_maps = []
    for c in range(NCORES):
        b, hh = c // 2, c % 2
        slab = np.zeros((IC, XP_FREE), np.float32)
        sh = xp[b, :, hh * SH_OUT_ROWS : hh * SH_OUT_ROWS + SH_ROWS, :]
        slab[:, : SH_ROWS * W132] = sh.reshape(IC, -1)
        m = {}
        if need_f32:
            m["xpf"] = slab
            m["wtsf"] = wts
        if need_bf16:
            m["xpb"] = slab.astype(ml_dtypes.bfloat16)
            m["wtsb"] = wtsb
        in_maps.append(m)
    return in_maps


def _ensure_ntff_hook():
    """The agent image's antenv lacks axon_hooks, so boot() could not register
    the NTFF profile hook. Inject the registry module and register the
    ctypes-based hook so run_bass_kernel_spmd(trace=True) can profile."""
    import types

    try:
        import antenv
    except ImportError:
        return
    if "antenv.axon_hooks" in sys.modules:
        return
    try:
        from trn_agent_boot.trn_boot import _ntff_profile_via_ctypes

        hook = _ntff_profile_via_ctypes("/opt/axon/libaxon_pjrt.so")
    except Exception:
        hook = None
    mod = types.ModuleType("antenv.axon_hooks")
    mod._hook = hook
    mod.set_axon_ntff_profile_hook = lambda h: setattr(mod, "_hook", h)
    mod.get_axon_ntff_profile_hook = lambda: mod._hook
    sys.modules["antenv.axon_hooks"] = mod
    antenv.axon_hooks = mod


def kernel(**inputs):
    global LAST_RESULT
    cfg = dict(CFG)
    in_maps = _host_prep(inputs, cfg)
    nc = _build(cfg)

    from concourse.bass_utils import run_bass_kernel_spmd

    trace = os.environ.get("KERNEL_TRACE", "0") == "1"
    if trace:
        _ensure_ntff_hook()
    res = run_bass_kernel_spmd(
        nc, in_maps, core_ids=list(range(NCORES)), trace=trace
    )
    LAST_RESULT = res

    out = np.empty((B, OC, H, W), np.float32)
    for c in range(NCORES):
        b, hh = c // 2, c % 2
        out[b, :, hh * SH_OUT_ROWS : (hh + 1) * SH_OUT_ROWS, :] = res.results[c]["out"]
    return out


# revision 55
# speedup vs baseline: 1.0551x; 1.0551x over previous
"""Trainium2 Bass kernel for AttentionStem (sparse local 4x4-window attention).

Contract: kernel(**inputs) takes the FULL unsharded inputs (numpy, keyed as in
setup_inputs()) and returns the FULL output [4, 64, 128, 128] float32.

Algorithm (per output pixel (b, h, w), per channel o):
    q  = query_w @ x                    (1x1 conv)
    kc = key_w @ xpad                   (1x1 conv on padded grid)
    vs_k = W_k @ xpad,  W_k = sum_m softmax_m(emb)[m,k] * value_w[m]
    attn_k = softmax_k(q * kc[pix + off_k])        (16 window positions)
    out = sum_k attn_k * vs_k[pix + off_k]

Sharding: 8 cores = 4 batches x 2 H-halves (64 output rows each, 4-row halo).
Layout on chip: [128 partitions = 2 row-blocks x 64 channels, free = pixels]
with padded row stride 132 so every window shift is a contiguous slice.
Softmax is computed without max subtraction: |logit| <= |q|*|k| stays well
below exp overflow for these Gaussian-scaled inputs.

Precision strategy: bf16 operands with fp32 PSUM accumulation on the
TensorEngine, bf16 on the 2x-mode VectorE path with split bf16 accumulator
slots across the 16 window positions (measured absmax error is dominated by
the value path, so bf16 logit convs cost nothing on this input). Window
positions are processed in vertical pairs (i, i+1) fused into single
VectorE/ScalarE ops via overlapping / broadcast access patterns.
Measured absmax rel err vs the fp32 reference: ~1.2e-2 (gate 2e-2).
Measured HW exec time: ~180 us on NeuronCore 0 (neuron-profile NTFF).
"""

import os
import sys

import numpy as np

sys.path.insert(0, "/opt/trn_rl_repo")

# Problem constants (hardcoded; kernel.py must be self-contained).
B, IC, OC, H, W = 4, 3, 64, 128, 128
KS, PAD, M = 4, 2, 4
NCORES = 8

W132 = W + 2 * PAD  # padded width = 132
SH_OUT_ROWS = 64  # output rows per core
SH_ROWS = SH_OUT_ROWS + KS  # padded input rows per core = 68
XP_FREE = SH_ROWS * W132 + 16  # xp slab free size (+pad for shifted reads)
BLK = 32  # output rows per partition-block
NBLK_FREE = BLK * W132  # 4224 free elems per block
KCV_ROWS = BLK + KS - 1  # 35 conv rows needed per block
KCV_FREE = KCV_ROWS * W132  # 4620
KCV_PAD = 16

# Config knobs (tuned on hardware).
CFG = {
    "conv_qk": os.environ.get("K_CONV_QK", "bf16"),  # logit conv matmul dtype
    "conv_vs": os.environ.get("K_CONV_VS", "bf16"),  # value conv matmul dtype
    "el": os.environ.get("K_EL", "bf16"),  # elementwise dtype (L/e/p/q/kcv)
    "acc": os.environ.get("K_ACC", "bf16"),  # s/num accumulator dtype
    "half": int(os.environ.get("K_HALF", "2")),  # column-split factor
    "gpadd": os.environ.get("K_GPADD", "1") == "1",  # split (ping-pong) accumulators
}

_CACHE = {}
LAST_RESULT = None  # BassKernelResults of the most recent run (for test.py)


def _dt(name):
    from concourse import mybir

    return {
        "f32": mybir.dt.float32,
        "f32r": mybir.dt.float32r,
        "bf16": mybir.dt.bfloat16,
    }[name]


def _emit(nc, tc, aps, cfg):
    """Emit the per-core program.

    aps: dict with xpf/xpb [3, XP_FREE], wtsf/wtsb [3, 64*18], out [64,64,128].
    wts layout: [qw.T | kw.T | W_0.T .. W_15.T]."""
    from contextlib import ExitStack

    import concourse.bass as bass
    from concourse import mybir

    f32 = mybir.dt.float32
    eldt = _dt(cfg["el"])
    adt = _dt(cfg["acc"])
    EXP = mybir.ActivationFunctionType.Exp
    NH = cfg["half"]
    HF = NBLK_FREE // NH  # elementwise free size per iteration
    el_bf16 = cfg["el"] == "bf16"

    with ExitStack() as ctx:
        const = ctx.enter_context(tc.tile_pool(name="const", bufs=1))
        qkp = ctx.enter_context(tc.tile_pool(name="qk", bufs=1))

        # ---- load inputs ----
        need_f32 = "f32" in (cfg["conv_qk"], cfg["conv_vs"]) or cfg["conv_qk"] == "f32r"
        need_bf16 = "bf16" in (cfg["conv_qk"], cfg["conv_vs"])
        xp = {}
        wts = {}
        # load order: weights, then the xp rows both row-blocks touch first
        # (block0 rows 0-23, block1 rows 32-55), then the remainder.
        P1A, P1B0, P1B1 = 12 * W132, 32 * W132, 44 * W132
        if need_f32:
            xp["f32"] = const.tile([IC, XP_FREE], f32, tag="xpf32", name="xpf32")
            wts["f32"] = const.tile([IC, OC * 18], f32, tag="wtsf32", name="wtsf32")
            xp["f32r"] = xp["f32"].bitcast(mybir.dt.float32r)
            wts["f32r"] = wts["f32"].bitcast(mybir.dt.float32r)
        if need_bf16:
            xp["bf16"] = const.tile([IC, XP_FREE], mybir.dt.bfloat16, tag="xpbf", name="xpbf")
            wts["bf16"] = const.tile([IC, OC * 18], mybir.dt.bfloat16, tag="wtsbf", name="wtsbf")
        # The logit-conv critical path (wtsf + first xpf pieces) gets the
        # sync queue with minimal descriptors ahead of it; the bf16 side
        # issues in parallel from the (idle at t=0) ScalarE/VectorE queues.
        if need_f32:
            nc.sync.dma_start(wts["f32"][:], aps["wtsf"][:])
        if need_bf16:
            nc.scalar.dma_start(wts["bf16"][:], aps["wtsb"][:])
        for lo, hi in ((0, P1A), (P1B0, P1B1), (P1A, P1B0), (P1B1, XP_FREE)):
            if need_f32:
                nc.sync.dma_start(xp["f32"][:, lo:hi], aps["xpf"][:, lo:hi])
            if need_bf16:
                nc.scalar.dma_start(xp["bf16"][:, lo:hi], aps["xpb"][:, lo:hi])

        def conv_rows(psum_pool, dst, wslot, dtname, base0, base1, total,
                      dst_off=0, chunk=1024, evac=None):
            """dst[128, f+dst_off] = sum_c w[o,c] * xp[c, base_blk + f]
            for partition 64*blk + o, f in [0, total). Chunked + evacuated."""
            xp_s = xp[dtname]
            w_l = wts[dtname][:, OC * wslot : OC * (wslot + 1)]
            off = 0
            while off < total:
                n = min(chunk, total - off)
                pt = psum_pool.tile([128, 1024], f32, tag="convp", name="cp")
                coff = 0
                while coff < n:
                    cn = min(512, n - coff)
                    for b, base in ((0, base0), (1, base1)):
                        nc.tensor.matmul(
                            pt[64 * b : 64 * (b + 1), coff : coff + cn],
                            w_l,
                            xp_s[:, base + off + coff : base + off + coff + cn],
                        )
                    coff += cn
                # evacuate PSUM -> SBUF (ScalarE: close to PSUM; casts dtype)
                (evac or nc.scalar.copy)(
                    dst[:, dst_off + off : dst_off + off + n], pt[:, :n]
                )
                off += n

        def conv_rows_compact(psum_pool, dst, wslot, dtname, base0, base1,
                              nrows_tot, dst_off=0):
            """Like conv_rows but skips the 4 pad columns per row: the matmul
            rhs uses a [rows,132->128] strided view, PSUM and dst stay
            contiguous at 128/row."""
            xp_s = xp[dtname]
            w_l = wts[dtname][:, OC * wslot : OC * (wslot + 1)]
            r = 0
            while r < nrows_tot:
                nr_c = min(8, nrows_tot - r)
                pt = psum_pool.tile([128, 1024], f32, tag="convp", name="cp")
                rr = 0
                while rr < nr_c:
                    sub = min(4, nr_c - rr)
                    for b, base in ((0, base0), (1, base1)):
                        rhs = xp_s[:, base + (r + rr) * W132 :
                                   base + (r + rr + sub) * W132]
                        rhs3 = rhs.rearrange("c (r w) -> c r w",
                                             w=W132)[:, :, 0:W]
                        nc.tensor.matmul(
                            pt[64 * b : 64 * (b + 1),
                               rr * W : (rr + sub) * W],
                            w_l, rhs3,
                        )
                    rr += sub
                nc.scalar.copy(
                    dst[:, dst_off + r * W : dst_off + (r + nr_c) * W],
                    pt[:, : nr_c * W],
                )
                r += nr_c

        # ---- q/kcv tiles (filled per column-half so the second half's convs
        # overlap the first half's k-loop instead of serializing up front) ----
        q = qkp.tile([128, NBLK_FREE], eldt, tag="q")
        # kcv with 1-element shifted twin so both shift parities have
        # 4B-aligned reads (keeps DVE 2x mode).
        kcv0 = qkp.tile([128, KCV_FREE + KCV_PAD], eldt, tag="kcv0")
        if el_bf16:
            kcv1 = qkp.tile([128, KCV_FREE + KCV_PAD], eldt, tag="kcv1")
        else:
            kcv1 = None
        # Sections of block-rows: a small first section so the first logit
        # multiply starts after ~1/4 of the phase-0 conv work; later sections'
        # q/kcv convs hide under the previous section's k-loop.
        # (row0, nrows, kcv_row_lo, kcv_row_hi)
        if NH == 2:
            SECTIONS = [(0, 4, 0, 8), (4, 12, 8, 20), (16, 16, 20, KCV_ROWS)]
        else:
            SECTIONS = [(0, BLK, 0, KCV_ROWS)]

        def qk_phase(sec, psum_pool):
            row0, nrows, klo_r, khi_r = SECTIONS[sec]
            qlo, qhi = row0 * W132, (row0 + nrows) * W132
            klo, khi = klo_r * W132, khi_r * W132
            ev = nc.vector.tensor_copy if sec == 0 else None
            # q: output rows b*32+h, cols w -> xp free (b*32+h+2)*132 + (w+2)
            conv_rows(
                psum_pool, q, 0, cfg["conv_qk"],
                (0 * BLK + PAD) * W132 + PAD + qlo,
                (1 * BLK + PAD) * W132 + PAD + qlo,
                qhi - qlo, dst_off=qlo, chunk=1024, evac=ev,
            )
            # kcv: conv at padded rows [b*32, b*32+35)
            conv_rows(
                psum_pool, kcv0, 1, cfg["conv_qk"],
                (0 * BLK) * W132 + klo,
                (1 * BLK) * W132 + klo,
                khi - klo, dst_off=klo, chunk=1024, evac=ev,
            )
            last = sec == len(SECTIONS) - 1
            if last:
                nc.vector.memset(kcv0[:, KCV_FREE:], 0.0)
            if kcv1 is not None:
                # kcv1[f] = kcv0[f+1]; section boundaries at khi*132 - 1
                lo = klo - 1 if klo > 0 else 0
                hi = (KCV_FREE + KCV_PAD - 8) if last else khi - 1
                if sec == 0:
                    # ramp window: VectorE is idle anyway
                    nc.vector.tensor_copy(kcv1[:, lo:hi], kcv0[:, lo + 1 : hi + 1])
                else:
                    # steady state: ride the idle DMA engines, split across
                    # partition quarters for bandwidth
                    for p0 in range(0, 128, 32):
                        nc.sync.dma_start(
                            kcv1[p0 : p0 + 32, lo:hi],
                            kcv0[p0 : p0 + 32, lo + 1 : hi + 1],
                        )
                if last:
                    nc.vector.memset(kcv1[:, KCV_FREE:], 0.0)

        # ---- phase 1: 16-way softmax-weighted accumulation ----
        # Software-pipelined: L_{k+1} is emitted while ACT runs exp_k so the
        # VectorE never waits on the ScalarE. e_k and p_k share one [128,2*HF]
        # tile (e left, p right) so the s/num accumulation is a single add.
        with ExitStack() as ctx1:
            vsp = ctx1.enter_context(tc.tile_pool(name="vs", bufs=2))
            psum1 = ctx1.enter_context(
                tc.tile_pool(name="psum1", bufs=4, space="PSUM")
            )
            epp = ctx1.enter_context(tc.tile_pool(name="ep", bufs=2))
            accp = ctx1.enter_context(tc.tile_pool(name="acc", bufs=2))
            finp = ctx1.enter_context(tc.tile_pool(name="fin", bufs=1))
            outp = ctx1.enter_context(tc.tile_pool(name="out", bufs=2))

            NK = KS * KS
            # vertical window pairs (i, j) & (i+1, j): kcv shifts differ by
            # exactly one padded row (132 elements - even, so bf16 alignment
            # and the 2x VectorE mode survive the paired access pattern)
            PAIRS = [(0, 4), (1, 5), (2, 6), (3, 7),
                     (8, 12), (9, 13), (10, 14), (11, 15)]

            def vs_pair(ka, kb, hoff, nrows):
                hfc = nrows * W  # compact (128/row) slot size
                v2 = vsp.tile([128, 2 * hfc], eldt, tag="vs", name="vs")
                for idx, k in ((0, ka), (1, kb)):
                    i, j = k // KS, k % KS
                    conv_rows_compact(
                        psum1, v2, 2 + k, cfg["conv_vs"],
                        (0 * BLK + i) * W132 + j + hoff,
                        (1 * BLK + i) * W132 + j + hoff,
                        nrows, dst_off=idx * hfc,
                    )
                return v2

            def sv(ap, off, nrows):
                """[128, nrows*132] slice viewed as [128, nrows, 128]: skips
                the 4 pad columns per row (3% fewer elements per DVE/ACT op;
                inner dim stays step-1 and 4B-aligned so 2x mode holds)."""
                v = ap[:, off : off + nrows * W132]
                return v.rearrange("p (r w) -> p r w", w=W132)[:, :, 0:W]

            def logit_pair(ka, hoff, hf, dst):
                """One op computes L_ka and L_{ka+4} into slots 0/1 of dst:
                the kcv operand uses a [132, 2] outer access dim (one row
                apart), the q operand a step-0 broadcast dim."""
                i, j = ka // KS, ka % KS
                shift = i * W132 + j
                if kcv1 is not None and (shift % 2) == 1:
                    ksrc, koff = kcv1, shift - 1
                else:
                    ksrc, koff = kcv0, shift
                nr = hf // W132
                vk = sv(ksrc, koff + hoff, nr)
                kk = bass.AP(vk.tensor, vk.offset,
                             [list(vk.ap)[0], [W132, 2], *list(vk.ap)[1:]])
                vq = sv(q, hoff, nr)
                qq = bass.AP(vq.tensor, vq.offset,
                             [list(vq.ap)[0], [0, 2], *list(vq.ap)[1:]])
                out = dst[:, 0 : 2 * hf].rearrange(
                    "p (s r w) -> p s r w", s=2, w=W132)[:, :, :, 0:W]
                nc.vector.tensor_mul(out, qq, kk)

            # Pair-fused k-loop: window positions processed two at a time in
            # one [128, 4*hf] tile laid out [e0|e1|p0|p1]. The logit multiply,
            # exp, value multiply, and accumulate each run once per pair
            # (half the per-op overheads), and the slot split doubles as the
            # accuracy "ping-pong".
            for sec in range(len(SECTIONS)):
                row0, nrows, _, _ = SECTIONS[sec]
                hoff, hf = row0 * W132, nrows * W132
                nr = nrows
                if sec == 0:
                    qk_phase(0, psum1)
                acc4 = accp.tile([128, 4 * hf], adt, tag="acc4", name="acc4")

                ep = epp.tile([128, 4 * hf], eldt, tag="ep", name="ep")
                vs2 = vs_pair(PAIRS[0][0], PAIRS[0][1], hoff, nrows)
                logit_pair(PAIRS[0][0], hoff, hf, ep)
                for t in range(NK // 2):
                    nc.scalar.activation(sv(ep, 0, 2 * nr), sv(ep, 0, 2 * nr),
                                         EXP)
                    nc.vector.tensor_mul(
                        sv(ep, 2 * hf, 2 * nr), sv(ep, 0, 2 * nr),
                        vs2[:, 0 : 2 * nr * W].rearrange(
                            "p (r w) -> p r w", w=W),
                    )
                    if t == (3 if sec == 0 else 6) and sec + 1 < len(SECTIONS):
                        # next section's q/kcv convs: emitted mid-loop so the
                        # TensorE has them ready before the section boundary
                        qk_phase(sec + 1, psum1)
                    ep_prev = ep
                    if t + 1 < NK // 2:
                        ep = epp.tile([128, 4 * hf], eldt, tag="ep", name="ep")
                        vs2 = vs_pair(PAIRS[t + 1][0], PAIRS[t + 1][1],
                                      hoff, nrows)
                        logit_pair(PAIRS[t + 1][0], hoff, hf, ep)
                    if t == 0:
                        nc.vector.tensor_copy(sv(acc4, 0, 4 * nr),
                                              sv(ep_prev, 0, 4 * nr))
                    else:
                        nc.vector.tensor_add(sv(acc4, 0, 4 * nr),
                                             sv(acc4, 0, 4 * nr),
                                             sv(ep_prev, 0, 4 * nr))

                # fold the even/odd slots: acc[s|num] = slots(0,2) + slots(1,3)
                acc = accp.tile([128, 2 * hf], adt, tag="acc", name="acc")
                a4 = acc4[:].rearrange("p (g s r w) -> p g s r w", g=2, s=2,
                                       w=W132)[:, :, :, :, 0:W]
                a2 = acc[:].rearrange("p (g r w) -> p g r w", g=2,
                                      w=W132)[:, :, :, 0:W]
                nc.vector.tensor_add(a2, a4[:, :, 0], a4[:, :, 1])

                # out = num / s  (s needs fp32 for the bit-level recip seed)
                # in column pieces so the store DMA overlaps the final mults;
                # compacted to the 128 valid columns per row
                npiece = 2 if nrows >= 8 else 1
                rpp = nrows // npiece  # block rows per final piece
                FP = rpp * W  # compact piece size
                for piece in range(npiece):
                    plo = piece * rpp * W132
                    s_f = finp.tile([128, FP], f32, tag="sf", name="sf")
                    nc.scalar.copy(s_f[:], sv(acc, plo, rpp))
                    rinv = finp.tile([128, FP], f32, tag="rinv", name="rinv")
                    nc.vector.reciprocal_approx_fast(rinv[:], s_f[:])
                    o_t = outp.tile([128, FP], f32, tag="o", name="o")
                    nc.vector.tensor_mul(
                        o_t[:], sv(acc, hf + plo, rpp),
                        rinv[:].rearrange("p (r w) -> p r w", w=W),
                    )
                    o_v = o_t[:].rearrange("p (r w) -> p r w", w=W)
                    rr = row0 + piece * rpp
                    for b in (0, 1):
                        nc.sync.dma_start(
                            aps["out"][:, b * BLK + rr : b * BLK + rr + rpp, :],
                            o_v[64 * b : 64 * (b + 1)],
                        )


def _build(cfg):
    key = tuple(sorted(cfg.items()))
    if key in _CACHE:
        return _CACHE[key]
    import concourse.tile as tile
    from concourse import bacc, mybir

    nc = bacc.Bacc(
        "TRN2", target_bir_lowering=False, debug=False, num_devices=NCORES
    )
    f32 = mybir.dt.float32
    bf16 = mybir.dt.bfloat16
    aps = {}
    need_f32 = "f32" in (cfg["conv_qk"], cfg["conv_vs"]) or cfg["conv_qk"] == "f32r"
    need_bf16 = "bf16" in (cfg["conv_qk"], cfg["conv_vs"])
    if need_f32:
        aps["xpf"] = nc.dram_tensor("xpf", [IC, XP_FREE], f32,
                                    kind="ExternalInput").ap()
        aps["wtsf"] = nc.dram_tensor("wtsf", [IC, OC * 18], f32,
                                     kind="ExternalInput").ap()
    if need_bf16:
        aps["xpb"] = nc.dram_tensor("xpb", [IC, XP_FREE], bf16,
                                    kind="ExternalInput").ap()
        aps["wtsb"] = nc.dram_tensor("wtsb", [IC, OC * 18], bf16,
                                     kind="ExternalInput").ap()
    aps["out"] = nc.dram_tensor("out", [OC, SH_OUT_ROWS, W], f32,
                                kind="ExternalOutput").ap()

    with tile.TileContext(nc) as tc:
        _emit(nc, tc, aps, cfg)
    nc.compile()
    _CACHE[key] = nc
    return nc


def _host_prep(inputs, cfg):
    import ml_dtypes

    x = np.asarray(inputs["x"], np.float32)
    key_w = np.asarray(inputs["key_w"], np.float32)
    query_w = np.asarray(inputs["query_w"], np.float32)
    value_w = np.asarray(inputs["value_w"], np.float32)
    emb_a = np.asarray(inputs["emb_a"], np.float32)
    emb_b = np.asarray(inputs["emb_b"], np.float32)
    emb_mix = np.asarray(inputs["emb_mix"], np.float32)

    # emb softmax over m, then effective per-offset value matrices W_k [16,64,3]
    la = emb_mix @ emb_a  # (M, KS)
    lb = emb_mix @ emb_b  # (M, KS)
    eloG = (la[:, :, None] + lb[:, None, :]).reshape(M, KS * KS).astype(np.float64)
    eloG -= eloG.max(axis=0, keepdims=True)
    emb = np.exp(eloG)
    emb /= emb.sum(axis=0, keepdims=True)  # (M, 16)
    wk = np.einsum("mk,moc->koc", emb.astype(np.float32), value_w)  # (16,64,3)

    # weights tensor [3, 64*18] = [qw.T | kw.T | W_0.T .. W_15.T]
    wts = np.empty((IC, OC * 18), np.float32)
    wts[:, 0:OC] = query_w.T
    wts[:, OC : 2 * OC] = key_w.T
    for k in range(KS * KS):
        wts[:, OC * (2 + k) : OC * (3 + k)] = wk[k].T

    # padded input, shards
    xp = np.zeros((B, IC, H + 2 * PAD, W + 2 * PAD), np.float32)
    xp[:, :, PAD : PAD + H, PAD : PAD + W] = x

    need_f32 = "f32" in (cfg["conv_qk"], cfg["conv_vs"]) or cfg["conv_qk"] == "f32r"
    need_bf16 = "bf16" in (cfg["conv_qk"], cfg["conv_vs"])
    wtsb = wts.astype(ml_dtypes.bfloat16)

    in_maps = []
    for c in range(NCORES):
        b, hh = c // 2, c % 2
        slab = np.zeros((IC, XP_FREE), np.float32)
        sh = xp[b, :, hh * SH_OUT_ROWS : hh * SH_OUT_ROWS + SH_ROWS, :]
        slab[:, : SH_ROWS * W132] = sh.reshape(IC, -1)
        m = {}
        if need_f32:
            m["xpf"] = slab
            m["wtsf"] = wts
        if need_bf16:
            m["xpb"] = slab.astype(ml_dtypes.bfloat16)
            m["wtsb"] = wtsb
        in_maps.append(m)
    return in_maps


def _ensure_ntff_hook():
    """The agent image's antenv lacks axon_hooks, so boot() could not register
    the NTFF profile hook. Inject the registry module and register the
    ctypes-based hook so run_bass_kernel_spmd(trace=True) can profile."""
    import types

    try:
        import antenv
    except ImportError:
        return
    if "antenv.axon_hooks" in sys.modules:
        return
    try:
        from trn_agent_boot.trn_boot import _ntff_profile_via_ctypes

        hook = _ntff_profile_via_ctypes("/opt/axon/libaxon_pjrt.so")
    except Exception:
        hook = None
    mod = types.ModuleType("antenv.axon_hooks")
    mod._hook = hook
    mod.set_axon_ntff_profile_hook = lambda h: setattr(mod, "_hook", h)
    mod.get_axon_ntff_profile_hook = lambda: mod._hook
    sys.modules["antenv.axon_hooks"] = mod
    antenv.axon_hooks = mod


def kernel(**inputs):
    global LAST_RESULT
    cfg = dict(CFG)
    in_maps = _host_prep(inputs, cfg)
    nc = _build(cfg)

    from concourse.bass_utils import run_bass_kernel_spmd

    trace = os.environ.get("KERNEL_TRACE", "0") == "1"
    if trace:
        _ensure_ntff_hook()
    res = run_bass_kernel_spmd(
        nc, in_maps, core_ids=list(range(NCORES)), trace=trace
    )
    LAST_RESULT = res

    out = np.empty((B, OC, H, W), np.float32)
    for c in range(NCORES):
        b, hh = c // 2, c % 2
        out[b, :, hh * SH_OUT_ROWS : (hh + 1) * SH_OUT_ROWS, :] = res.results[c]["out"]
    return out


# revision 56
# speedup vs baseline: 1.0564x; 1.0013x over previous
"""Trainium2 Bass kernel for AttentionStem (sparse local 4x4-window attention).

Contract: kernel(**inputs) takes the FULL unsharded inputs (numpy, keyed as in
setup_inputs()) and returns the FULL output [4, 64, 128, 128] float32.

Algorithm (per output pixel (b, h, w), per channel o):
    q  = query_w @ x                    (1x1 conv)
    kc = key_w @ xpad                   (1x1 conv on padded grid)
    vs_k = W_k @ xpad,  W_k = sum_m softmax_m(emb)[m,k] * value_w[m]
    attn_k = softmax_k(q * kc[pix + off_k])        (16 window positions)
    out = sum_k attn_k * vs_k[pix + off_k]

Sharding: 8 cores = 4 batches x 2 H-halves (64 output rows each, 4-row halo).
Layout on chip: [128 partitions = 2 row-blocks x 64 channels, free = pixels]
with padded row stride 132 so every window shift is a contiguous slice.
Softmax is computed without max subtraction: |logit| <= |q|*|k| stays well
below exp overflow for these Gaussian-scaled inputs.

Precision strategy: bf16 operands with fp32 PSUM accumulation on the
TensorEngine, bf16 on the 2x-mode VectorE path with split bf16 accumulator
slots across the 16 window positions (measured absmax error is dominated by
the value path, so bf16 logit convs cost nothing on this input). Window
positions are processed in vertical pairs (i, i+1) fused into single
VectorE/ScalarE ops via overlapping / broadcast access patterns.
Measured absmax rel err vs the fp32 reference: ~1.2e-2 (gate 2e-2).
Measured HW exec time: ~180 us on NeuronCore 0 (neuron-profile NTFF).
"""

import os
import sys

import numpy as np

sys.path.insert(0, "/opt/trn_rl_repo")

# Problem constants (hardcoded; kernel.py must be self-contained).
B, IC, OC, H, W = 4, 3, 64, 128, 128
KS, PAD, M = 4, 2, 4
NCORES = 8

W132 = W + 2 * PAD  # padded width = 132
SH_OUT_ROWS = 64  # output rows per core
SH_ROWS = SH_OUT_ROWS + KS  # padded input rows per core = 68
XP_FREE = SH_ROWS * W132 + 16  # xp slab free size (+pad for shifted reads)
BLK = 32  # output rows per partition-block
NBLK_FREE = BLK * W132  # 4224 free elems per block
KCV_ROWS = BLK + KS - 1  # 35 conv rows needed per block
KCV_FREE = KCV_ROWS * W132  # 4620
KCV_PAD = 16

# Config knobs (tuned on hardware).
CFG = {
    "conv_qk": os.environ.get("K_CONV_QK", "bf16"),  # logit conv matmul dtype
    "conv_vs": os.environ.get("K_CONV_VS", "bf16"),  # value conv matmul dtype
    "el": os.environ.get("K_EL", "bf16"),  # elementwise dtype (L/e/p/q/kcv)
    "acc": os.environ.get("K_ACC", "bf16"),  # s/num accumulator dtype
    "half": int(os.environ.get("K_HALF", "2")),  # column-split factor
    "gpadd": os.environ.get("K_GPADD", "1") == "1",  # split (ping-pong) accumulators
}

_CACHE = {}
LAST_RESULT = None  # BassKernelResults of the most recent run (for test.py)


def _dt(name):
    from concourse import mybir

    return {
        "f32": mybir.dt.float32,
        "f32r": mybir.dt.float32r,
        "bf16": mybir.dt.bfloat16,
    }[name]


def _emit(nc, tc, aps, cfg):
    """Emit the per-core program.

    aps: dict with xpf/xpb [3, XP_FREE], wtsf/wtsb [3, 64*18], out [64,64,128].
    wts layout: [qw.T | kw.T | W_0.T .. W_15.T]."""
    from contextlib import ExitStack

    import concourse.bass as bass
    from concourse import mybir

    f32 = mybir.dt.float32
    eldt = _dt(cfg["el"])
    adt = _dt(cfg["acc"])
    EXP = mybir.ActivationFunctionType.Exp
    NH = cfg["half"]
    HF = NBLK_FREE // NH  # elementwise free size per iteration
    el_bf16 = cfg["el"] == "bf16"

    with ExitStack() as ctx:
        const = ctx.enter_context(tc.tile_pool(name="const", bufs=1))
        qkp = ctx.enter_context(tc.tile_pool(name="qk", bufs=1))

        # ---- load inputs ----
        need_f32 = "f32" in (cfg["conv_qk"], cfg["conv_vs"]) or cfg["conv_qk"] == "f32r"
        need_bf16 = "bf16" in (cfg["conv_qk"], cfg["conv_vs"])
        xp = {}
        wts = {}
        # load order: weights, then the xp rows both row-blocks touch first
        # (block0 rows 0-23, block1 rows 32-55), then the remainder.
        P1A, P1B0, P1B1 = 12 * W132, 32 * W132, 44 * W132
        if need_f32:
            xp["f32"] = const.tile([IC, XP_FREE], f32, tag="xpf32", name="xpf32")
            wts["f32"] = const.tile([IC, OC * 18], f32, tag="wtsf32", name="wtsf32")
            xp["f32r"] = xp["f32"].bitcast(mybir.dt.float32r)
            wts["f32r"] = wts["f32"].bitcast(mybir.dt.float32r)
        if need_bf16:
            xp["bf16"] = const.tile([IC, XP_FREE], mybir.dt.bfloat16, tag="xpbf", name="xpbf")
            wts["bf16"] = const.tile([IC, OC * 18], mybir.dt.bfloat16, tag="wtsbf", name="wtsbf")
        # The logit-conv critical path (wtsf + first xpf pieces) gets the
        # sync queue with minimal descriptors ahead of it; the bf16 side
        # issues in parallel from the (idle at t=0) ScalarE/VectorE queues.
        if need_f32:
            nc.sync.dma_start(wts["f32"][:], aps["wtsf"][:])
        if need_bf16:
            nc.scalar.dma_start(wts["bf16"][:], aps["wtsb"][:])
        first = True
        for lo, hi in ((0, P1A), (P1B0, P1B1), (P1A, P1B0), (P1B1, XP_FREE)):
            if need_f32:
                nc.sync.dma_start(xp["f32"][:, lo:hi], aps["xpf"][:, lo:hi])
            if need_bf16:
                # first piece on the otherwise-empty sync queue so its
                # descriptor doesn't wait behind the weights DMA
                eng = nc.sync if first and not need_f32 else nc.scalar
                eng.dma_start(xp["bf16"][:, lo:hi], aps["xpb"][:, lo:hi])
            first = False

        def conv_rows(psum_pool, dst, wslot, dtname, base0, base1, total,
                      dst_off=0, chunk=1024, evac=None):
            """dst[128, f+dst_off] = sum_c w[o,c] * xp[c, base_blk + f]
            for partition 64*blk + o, f in [0, total). Chunked + evacuated."""
            xp_s = xp[dtname]
            w_l = wts[dtname][:, OC * wslot : OC * (wslot + 1)]
            off = 0
            while off < total:
                n = min(chunk, total - off)
                pt = psum_pool.tile([128, 1024], f32, tag="convp", name="cp")
                coff = 0
                while coff < n:
                    cn = min(512, n - coff)
                    for b, base in ((0, base0), (1, base1)):
                        nc.tensor.matmul(
                            pt[64 * b : 64 * (b + 1), coff : coff + cn],
                            w_l,
                            xp_s[:, base + off + coff : base + off + coff + cn],
                        )
                    coff += cn
                # evacuate PSUM -> SBUF (ScalarE: close to PSUM; casts dtype)
                (evac or nc.scalar.copy)(
                    dst[:, dst_off + off : dst_off + off + n], pt[:, :n]
                )
                off += n

        def conv_rows_compact(psum_pool, dst, wslot, dtname, base0, base1,
                              nrows_tot, dst_off=0):
            """Like conv_rows but skips the 4 pad columns per row: the matmul
            rhs uses a [rows,132->128] strided view, PSUM and dst stay
            contiguous at 128/row."""
            xp_s = xp[dtname]
            w_l = wts[dtname][:, OC * wslot : OC * (wslot + 1)]
            r = 0
            while r < nrows_tot:
                nr_c = min(8, nrows_tot - r)
                pt = psum_pool.tile([128, 1024], f32, tag="convp", name="cp")
                rr = 0
                while rr < nr_c:
                    sub = min(4, nr_c - rr)
                    for b, base in ((0, base0), (1, base1)):
                        rhs = xp_s[:, base + (r + rr) * W132 :
                                   base + (r + rr + sub) * W132]
                        rhs3 = rhs.rearrange("c (r w) -> c r w",
                                             w=W132)[:, :, 0:W]
                        nc.tensor.matmul(
                            pt[64 * b : 64 * (b + 1),
                               rr * W : (rr + sub) * W],
                            w_l, rhs3,
                        )
                    rr += sub
                nc.scalar.copy(
                    dst[:, dst_off + r * W : dst_off + (r + nr_c) * W],
                    pt[:, : nr_c * W],
                )
                r += nr_c

        # ---- q/kcv tiles (filled per column-half so the second half's convs
        # overlap the first half's k-loop instead of serializing up front) ----
        q = qkp.tile([128, NBLK_FREE], eldt, tag="q")
        # kcv with 1-element shifted twin so both shift parities have
        # 4B-aligned reads (keeps DVE 2x mode).
        kcv0 = qkp.tile([128, KCV_FREE + KCV_PAD], eldt, tag="kcv0")
        if el_bf16:
            kcv1 = qkp.tile([128, KCV_FREE + KCV_PAD], eldt, tag="kcv1")
        else:
            kcv1 = None
        # Sections of block-rows: a small first section so the first logit
        # multiply starts after ~1/4 of the phase-0 conv work; later sections'
        # q/kcv convs hide under the previous section's k-loop.
        # (row0, nrows, kcv_row_lo, kcv_row_hi)
        if NH == 2:
            SECTIONS = [(0, 4, 0, 8), (4, 12, 8, 20), (16, 16, 20, KCV_ROWS)]
        else:
            SECTIONS = [(0, BLK, 0, KCV_ROWS)]

        def qk_phase(sec, psum_pool):
            row0, nrows, klo_r, khi_r = SECTIONS[sec]
            qlo, qhi = row0 * W132, (row0 + nrows) * W132
            klo, khi = klo_r * W132, khi_r * W132
            ev = nc.vector.tensor_copy if sec == 0 else None
            # q: output rows b*32+h, cols w -> xp free (b*32+h+2)*132 + (w+2)
            conv_rows(
                psum_pool, q, 0, cfg["conv_qk"],
                (0 * BLK + PAD) * W132 + PAD + qlo,
                (1 * BLK + PAD) * W132 + PAD + qlo,
                qhi - qlo, dst_off=qlo, chunk=1024, evac=ev,
            )
            # kcv: conv at padded rows [b*32, b*32+35)
            conv_rows(
                psum_pool, kcv0, 1, cfg["conv_qk"],
                (0 * BLK) * W132 + klo,
                (1 * BLK) * W132 + klo,
                khi - klo, dst_off=klo, chunk=1024, evac=ev,
            )
            last = sec == len(SECTIONS) - 1
            if last:
                nc.vector.memset(kcv0[:, KCV_FREE:], 0.0)
            if kcv1 is not None:
                # kcv1[f] = kcv0[f+1]; section boundaries at khi*132 - 1
                lo = klo - 1 if klo > 0 else 0
                hi = (KCV_FREE + KCV_PAD - 8) if last else khi - 1
                if sec == 0:
                    # ramp window: VectorE is idle anyway
                    nc.vector.tensor_copy(kcv1[:, lo:hi], kcv0[:, lo + 1 : hi + 1])
                else:
                    # steady state: ride the idle DMA engines, split across
                    # partition quarters for bandwidth
                    for p0 in range(0, 128, 32):
                        nc.sync.dma_start(
                            kcv1[p0 : p0 + 32, lo:hi],
                            kcv0[p0 : p0 + 32, lo + 1 : hi + 1],
                        )
                if last:
                    nc.vector.memset(kcv1[:, KCV_FREE:], 0.0)

        # ---- phase 1: 16-way softmax-weighted accumulation ----
        # Software-pipelined: L_{k+1} is emitted while ACT runs exp_k so the
        # VectorE never waits on the ScalarE. e_k and p_k share one [128,2*HF]
        # tile (e left, p right) so the s/num accumulation is a single add.
        with ExitStack() as ctx1:
            vsp = ctx1.enter_context(tc.tile_pool(name="vs", bufs=2))
            psum1 = ctx1.enter_context(
                tc.tile_pool(name="psum1", bufs=4, space="PSUM")
            )
            epp = ctx1.enter_context(tc.tile_pool(name="ep", bufs=2))
            accp = ctx1.enter_context(tc.tile_pool(name="acc", bufs=2))
            finp = ctx1.enter_context(tc.tile_pool(name="fin", bufs=1))
            outp = ctx1.enter_context(tc.tile_pool(name="out", bufs=2))

            NK = KS * KS
            # vertical window pairs (i, j) & (i+1, j): kcv shifts differ by
            # exactly one padded row (132 elements - even, so bf16 alignment
            # and the 2x VectorE mode survive the paired access pattern)
            PAIRS = [(0, 4), (1, 5), (2, 6), (3, 7),
                     (8, 12), (9, 13), (10, 14), (11, 15)]

            def vs_pair(ka, kb, hoff, nrows):
                hfc = nrows * W  # compact (128/row) slot size
                v2 = vsp.tile([128, 2 * hfc], eldt, tag="vs", name="vs")
                for idx, k in ((0, ka), (1, kb)):
                    i, j = k // KS, k % KS
                    conv_rows_compact(
                        psum1, v2, 2 + k, cfg["conv_vs"],
                        (0 * BLK + i) * W132 + j + hoff,
                        (1 * BLK + i) * W132 + j + hoff,
                        nrows, dst_off=idx * hfc,
                    )
                return v2

            def sv(ap, off, nrows):
                """[128, nrows*132] slice viewed as [128, nrows, 128]: skips
                the 4 pad columns per row (3% fewer elements per DVE/ACT op;
                inner dim stays step-1 and 4B-aligned so 2x mode holds)."""
                v = ap[:, off : off + nrows * W132]
                return v.rearrange("p (r w) -> p r w", w=W132)[:, :, 0:W]

            def logit_pair(ka, hoff, hf, dst):
                """One op computes L_ka and L_{ka+4} into slots 0/1 of dst:
                the kcv operand uses a [132, 2] outer access dim (one row
                apart), the q operand a step-0 broadcast dim."""
                i, j = ka // KS, ka % KS
                shift = i * W132 + j
                if kcv1 is not None and (shift % 2) == 1:
                    ksrc, koff = kcv1, shift - 1
                else:
                    ksrc, koff = kcv0, shift
                nr = hf // W132
                vk = sv(ksrc, koff + hoff, nr)
                kk = bass.AP(vk.tensor, vk.offset,
                             [list(vk.ap)[0], [W132, 2], *list(vk.ap)[1:]])
                vq = sv(q, hoff, nr)
                qq = bass.AP(vq.tensor, vq.offset,
                             [list(vq.ap)[0], [0, 2], *list(vq.ap)[1:]])
                out = dst[:, 0 : 2 * hf].rearrange(
                    "p (s r w) -> p s r w", s=2, w=W132)[:, :, :, 0:W]
                nc.vector.tensor_mul(out, qq, kk)

            # Pair-fused k-loop: window positions processed two at a time in
            # one [128, 4*hf] tile laid out [e0|e1|p0|p1]. The logit multiply,
            # exp, value multiply, and accumulate each run once per pair
            # (half the per-op overheads), and the slot split doubles as the
            # accuracy "ping-pong".
            for sec in range(len(SECTIONS)):
                row0, nrows, _, _ = SECTIONS[sec]
                hoff, hf = row0 * W132, nrows * W132
                nr = nrows
                if sec == 0:
                    qk_phase(0, psum1)
                acc4 = accp.tile([128, 4 * hf], adt, tag="acc4", name="acc4")

                ep = epp.tile([128, 4 * hf], eldt, tag="ep", name="ep")
                vs2 = vs_pair(PAIRS[0][0], PAIRS[0][1], hoff, nrows)
                logit_pair(PAIRS[0][0], hoff, hf, ep)
                for t in range(NK // 2):
                    nc.scalar.activation(sv(ep, 0, 2 * nr), sv(ep, 0, 2 * nr),
                                         EXP)
                    nc.vector.tensor_mul(
                        sv(ep, 2 * hf, 2 * nr), sv(ep, 0, 2 * nr),
                        vs2[:, 0 : 2 * nr * W].rearrange(
                            "p (r w) -> p r w", w=W),
                    )
                    if t == (3 if sec == 0 else 6) and sec + 1 < len(SECTIONS):
                        # next section's q/kcv convs: emitted mid-loop so the
                        # TensorE has them ready before the section boundary
                        qk_phase(sec + 1, psum1)
                    ep_prev = ep
                    if t + 1 < NK // 2:
                        ep = epp.tile([128, 4 * hf], eldt, tag="ep", name="ep")
                        vs2 = vs_pair(PAIRS[t + 1][0], PAIRS[t + 1][1],
                                      hoff, nrows)
                        logit_pair(PAIRS[t + 1][0], hoff, hf, ep)
                    if t == 0:
                        nc.vector.tensor_copy(sv(acc4, 0, 4 * nr),
                                              sv(ep_prev, 0, 4 * nr))
                    else:
                        nc.vector.tensor_add(sv(acc4, 0, 4 * nr),
                                             sv(acc4, 0, 4 * nr),
                                             sv(ep_prev, 0, 4 * nr))

                # fold the even/odd slots: acc[s|num] = slots(0,2) + slots(1,3)
                acc = accp.tile([128, 2 * hf], adt, tag="acc", name="acc")
                a4 = acc4[:].rearrange("p (g s r w) -> p g s r w", g=2, s=2,
                                       w=W132)[:, :, :, :, 0:W]
                a2 = acc[:].rearrange("p (g r w) -> p g r w", g=2,
                                      w=W132)[:, :, :, 0:W]
                nc.vector.tensor_add(a2, a4[:, :, 0], a4[:, :, 1])

                # out = num / s  (s needs fp32 for the bit-level recip seed)
                # in column pieces so the store DMA overlaps the final mults;
                # compacted to the 128 valid columns per row
                npiece = 2 if nrows >= 8 else 1
                rpp = nrows // npiece  # block rows per final piece
                FP = rpp * W  # compact piece size
                for piece in range(npiece):
                    plo = piece * rpp * W132
                    s_f = finp.tile([128, FP], f32, tag="sf", name="sf")
                    nc.scalar.copy(s_f[:], sv(acc, plo, rpp))
                    rinv = finp.tile([128, FP], f32, tag="rinv", name="rinv")
                    nc.vector.reciprocal_approx_fast(rinv[:], s_f[:])
                    o_t = outp.tile([128, FP], f32, tag="o", name="o")
                    nc.vector.tensor_mul(
                        o_t[:], sv(acc, hf + plo, rpp),
                        rinv[:].rearrange("p (r w) -> p r w", w=W),
                    )
                    o_v = o_t[:].rearrange("p (r w) -> p r w", w=W)
                    rr = row0 + piece * rpp
                    for b in (0, 1):
                        nc.sync.dma_start(
                            aps["out"][:, b * BLK + rr : b * BLK + rr + rpp, :],
                            o_v[64 * b : 64 * (b + 1)],
                        )


def _build(cfg):
    key = tuple(sorted(cfg.items()))
    if key in _CACHE:
        return _CACHE[key]
    import concourse.tile as tile
    from concourse import bacc, mybir

    nc = bacc.Bacc(
        "TRN2", target_bir_lowering=False, debug=False, num_devices=NCORES
    )
    f32 = mybir.dt.float32
    bf16 = mybir.dt.bfloat16
    aps = {}
    need_f32 = "f32" in (cfg["conv_qk"], cfg["conv_vs"]) or cfg["conv_qk"] == "f32r"
    need_bf16 = "bf16" in (cfg["conv_qk"], cfg["conv_vs"])
    if need_f32:
        aps["xpf"] = nc.dram_tensor("xpf", [IC, XP_FREE], f32,
                                    kind="ExternalInput").ap()
        aps["wtsf"] = nc.dram_tensor("wtsf", [IC, OC * 18], f32,
                                     kind="ExternalInput").ap()
    if need_bf16:
        aps["xpb"] = nc.dram_tensor("xpb", [IC, XP_FREE], bf16,
                                    kind="ExternalInput").ap()
        aps["wtsb"] = nc.dram_tensor("wtsb", [IC, OC * 18], bf16,
                                     kind="ExternalInput").ap()
    aps["out"] = nc.dram_tensor("out", [OC, SH_OUT_ROWS, W], f32,
                                kind="ExternalOutput").ap()

    with tile.TileContext(nc) as tc:
        _emit(nc, tc, aps, cfg)
    nc.compile()
    _CACHE[key] = nc
    return nc


def _host_prep(inputs, cfg):
    import ml_dtypes

    x = np.asarray(inputs["x"], np.float32)
    key_w = np.asarray(inputs["key_w"], np.float32)
    query_w = np.asarray(inputs["query_w"], np.float32)
    value_w = np.asarray(inputs["value_w"], np.float32)
    emb_a = np.asarray(inputs["emb_a"], np.float32)
    emb_b = np.asarray(inputs["emb_b"], np.float32)
    emb_mix = np.asarray(inputs["emb_mix"], np.float32)

    # emb softmax over m, then effective per-offset value matrices W_k [16,64,3]
    la = emb_mix @ emb_a  # (M, KS)
    lb = emb_mix @ emb_b  # (M, KS)
    eloG = (la[:, :, None] + lb[:, None, :]).reshape(M, KS * KS).astype(np.float64)
    eloG -= eloG.max(axis=0, keepdims=True)
    emb = np.exp(eloG)
    emb /= emb.sum(axis=0, keepdims=True)  # (M, 16)
    wk = np.einsum("mk,moc->koc", emb.astype(np.float32), value_w)  # (16,64,3)

    # weights tensor [3, 64*18] = [qw.T | kw.T | W_0.T .. W_15.T]
    wts = np.empty((IC, OC * 18), np.float32)
    wts[:, 0:OC] = query_w.T
    wts[:, OC : 2 * OC] = key_w.T
    for k in range(KS * KS):
        wts[:, OC * (2 + k) : OC * (3 + k)] = wk[k].T

    # padded input, shards
    xp = np.zeros((B, IC, H + 2 * PAD, W + 2 * PAD), np.float32)
    xp[:, :, PAD : PAD + H, PAD : PAD + W] = x

    need_f32 = "f32" in (cfg["conv_qk"], cfg["conv_vs"]) or cfg["conv_qk"] == "f32r"
    need_bf16 = "bf16" in (cfg["conv_qk"], cfg["conv_vs"])
    wtsb = wts.astype(ml_dtypes.bfloat16)

    in_maps = []
    for c in range(NCORES):
        b, hh = c // 2, c % 2
        slab = np.zeros((IC, XP_FREE), np.float32)
        sh = xp[b, :, hh * SH_OUT_ROWS : hh * SH_OUT_ROWS + SH_ROWS, :]
        slab[:, : SH_ROWS * W132] = sh.reshape(IC, -1)
        m = {}
        if need_f32:
            m["xpf"] = slab
            m["wtsf"] = wts
        if need_bf16:
            m["xpb"] = slab.astype(ml_dtypes.bfloat16)
            m["wtsb"] = wtsb
        in_maps.append(m)
    return in_maps


def _ensure_ntff_hook():
    """The agent image's antenv lacks axon_hooks, so boot() could not register
    the NTFF profile hook. Inject the registry module and register the
    ctypes-based hook so run_bass_kernel_spmd(trace=True) can profile."""
    import types

    try:
        import antenv
    except ImportError:
        return
    if "antenv.axon_hooks" in sys.modules:
        return
    try:
        from trn_agent_boot.trn_boot import _ntff_profile_via_ctypes

        hook = _ntff_profile_via_ctypes("/opt/axon/libaxon_pjrt.so")
    except Exception:
        hook = None
    mod = types.ModuleType("antenv.axon_hooks")
    mod._hook = hook
    mod.set_axon_ntff_profile_hook = lambda h: setattr(mod, "_hook", h)
    mod.get_axon_ntff_profile_hook = lambda: mod._hook
    sys.modules["antenv.axon_hooks"] = mod
    antenv.axon_hooks = mod


def kernel(**inputs):
    global LAST_RESULT
    cfg = dict(CFG)
    in_maps = _host_prep(inputs, cfg)
    nc = _build(cfg)

    from concourse.bass_utils import run_bass_kernel_spmd

    trace = os.environ.get("KERNEL_TRACE", "0") == "1"
    if trace:
        _ensure_ntff_hook()
    res = run_bass_kernel_spmd(
        nc, in_maps, core_ids=list(range(NCORES)), trace=trace
    )
    LAST_RESULT = res

    out = np.empty((B, OC, H, W), np.float32)
    for c in range(NCORES):
        b, hh = c // 2, c % 2
        out[b, :, hh * SH_OUT_ROWS : (hh + 1) * SH_OUT_ROWS, :] = res.results[c]["out"]
    return out
